# revision 17
# baseline (speedup 1.0000x reference)
"""nn_ContrastiveLoss Trainium2 kernel v2 (8 NeuronCores, data-parallel batch).

Contract: kernel(embeddings=[64,1024,128] f32, labels=[64,1024] int64) -> f32.

Host-side (sharding/layout only, no loss arithmetic):
  - per sample: rows reordered pos-first / neg-second, transposed to [D, rows],
    cast bf16, zero-padded to slot shapes.
  - slot-sorting: the 64 samples are ranked by npos; slot k of every core gets
    one of ranks [8k, 8k+8), so the SPMD program's per-slot static shapes
    (nA = padded pos cols, nB = neg cols) stay tight across cores.
  - final combine: sums device partials, applies the exact max-trick
    correction, divides by per-sample neg counts and the global count
    (the all-reduce + division of the sharding hint).

Device per slot (sample):
  - DMA transposed bf16 tile ett [128d, L]  (A=pos cols | B=neg cols)
  - sq = ett*ett (DVE bf16 2x)
  - norms via PE: col-tiled ones-matmuls fold nsq into a [128, 384] psum tile
  - r = sqrt(nsq + eps^2) (ACT), rinv = reciprocal_approx_fast (DVE)
  - rinv row assembled on partition 0 (DMA), nat-layout rinv/thresh for the
    per-partition hinge scales (DMA + tiny DVE ops)
  - rinv broadcast across partitions (GpSimd daisy chain), ets_B = et_B*rinv
  - sim = et_A_chunk^T @ ets_B (PE bf16, exact nB width)
  - hinge reduce per chunk, alternating engines:
      ACT: relu(rinv_i * sim - t) with accum_out
      DVE: max(sim, t*r_i) with accum_out (host unscales by rinv_i exactly)
"""

import sys

if "/opt/trn_rl_repo" not in sys.path:
    sys.path.insert(0, "/opt/trn_rl_repo")

from contextlib import ExitStack

import numpy as np
import ml_dtypes

import concourse.bass as bass
import concourse.bacc as bacc
import concourse.mybir as mybir
import concourse.tile as tile
from concourse import bass_utils

F32 = mybir.dt.float32
BF16 = mybir.dt.bfloat16
FP16 = mybir.dt.float16
AF = mybir.ActivationFunctionType
ALU = mybir.AluOpType

P = 128
D = 128
N = 1024
B = 64
NCORES = 8
BPC = B // NCORES
THRESH = 0.5 - 0.35          # 0.15
EPS = 1e-4
WG = 384                     # fold group width (3 x 128)
CNAT = 5                     # nat-layout rinv columns per slot
DVE_EVERY = 3                # every DVE_EVERY-th hinge chunk goes to DVE


def _kernel_body(ctx, tc, ett_ap, out_ap, rout_ap, meta):
    nc = tc.nc
    slots, lmax, c_total = meta["slots"], meta["lmax"], meta["c_total"]

    const_pool = ctx.enter_context(tc.tile_pool(name="const", bufs=1))
    epool = ctx.enter_context(tc.tile_pool(name="epool", bufs=BPC))
    sqpool = ctx.enter_context(tc.tile_pool(name="sqpool", bufs=3))
    rpool = ctx.enter_context(tc.tile_pool(name="rpool", bufs=1))
    bpool = ctx.enter_context(tc.tile_pool(name="bpool", bufs=8))
    fold_psum = ctx.enter_context(tc.tile_pool(name="foldps", bufs=3, space="PSUM"))
    tr_psum = ctx.enter_context(tc.tile_pool(name="trps", bufs=1, space="PSUM"))
    sim_psum = ctx.enter_context(tc.tile_pool(name="simps", bufs=2, space="PSUM"))

    ones32 = const_pool.tile([P, 32], BF16)
    nc.gpsimd.memset(ones32[:], 1.0)
    eps2 = const_pool.tile([P, 1], F32)
    nc.gpsimd.memset(eps2[:], EPS * EPS)
    negt = const_pool.tile([P, 1], F32)
    nc.gpsimd.memset(negt[:], -THRESH)

    onesh = const_pool.tile([P, 1], FP16)
    nc.gpsimd.memset(onesh[:], 1.0)
    ident = const_pool.tile([P, P], FP16)
    nc.gpsimd.affine_select(ident[:], onesh[:].broadcast_to([P, P]),
                            pattern=[[-1, P]], compare_op=ALU.is_equal,
                            fill=0.0, base=0, channel_multiplier=1)

    # ACT table warmup (sqrt set includes relu) off the critical path
    warm = const_pool.tile([P, 1], F32)
    nc.scalar.activation(warm[:], eps2[:], AF.Sqrt, bias=eps2[:])
    nc.scalar.activation(warm[:], warm[:], AF.Relu, bias=0.0)
    # gpsimd ext-isa warmup: pulls the ~6-9us IRAM lib load off the
    # critical path (partition_broadcast is a loadable Q7 kernel)
    warmb = const_pool.tile([P, 2], FP16)
    nc.gpsimd.partition_broadcast(warmb[:], onesh[0:1, :].broadcast_to([1, 2]),
                                  channels=P)
    # PE HAM warmup: ~5us of dummy matmuls while the input DMAs land, so the
    # PE clock gate is already at 8/8 when the first fold/sim matmuls issue
    warm_src = const_pool.tile([P, 512], BF16)
    nc.vector.memset(warm_src[:], 0.0)
    warm_ps = fold_psum.tile([P, WG], F32, tag="fold")
    for _ in range(16):
        nc.tensor.matmul(warm_ps[0:32, 0:WG], lhsT=ones32[:],
                         rhs=warm_src[:, 0:WG], start=True, stop=True)

    # persistent tiles
    rinv_f4_all = rpool.tile([P, BPC, WG], F32)        # folded rinv, 4 strips
    rinv_f4_h = rpool.tile([P, BPC, WG], FP16)         # fp16 copy
    row_all = rpool.tile([1, BPC, 4 * WG], FP16)       # rinv rows (partition 0)
    aux = rpool.tile([P, 2, BPC * CNAT], F32)          # f32 rinv / thresh
    slot_all = rpool.tile([P, c_total], F32)           # hinge partials

    # defined values for unused strips/cols read by batched DMAs/ops
    nc.gpsimd.memset(rinv_f4_h[:], 1.0)
    nc.vector.memset(aux[:], 1.0)

    ett_tiles = [None] * BPC

    def load_slot(k):
        s = slots[k]
        ett = epool.tile([P, lmax], BF16, tag="ett")
        ett_tiles[k] = ett
        nc.sync.dma_start(ett[:, 0:s["lpad"]],
                          ett_ap[:, s["off"]:s["off"] + s["lpad"]])

    fold_tiles = [None] * BPC

    def sq_fold_slot(k):
        s = slots[k]
        lpad = s["lpad"]
        gcnt = lpad // WG
        ett = ett_tiles[k]
        sq = sqpool.tile([P, lmax], BF16, tag="sq")
        nc.vector.tensor_tensor(sq[:, 0:lpad], ett[:, 0:lpad], ett[:, 0:lpad],
                                ALU.mult)
        fold = fold_psum.tile([P, WG], F32, tag="fold")
        fold_tiles[k] = fold
        for g in range(gcnt):
            nc.tensor.matmul(fold[32 * g:32 * g + 32, :],
                             lhsT=ones32[:], rhs=sq[:, g * WG:(g + 1) * WG],
                             start=True, stop=True, tile_position=(0, 32 * g))

    def phase_a(k):
        s = slots[k]
        cA, nB = s["cA"], s["nB"]
        lpad = s["lpad"]
        gcnt = lpad // WG
        fold = fold_tiles[k]

        gp = 32 * gcnt
        r4 = rinv_f4_all[:, k, :]
        nc.scalar.activation(r4[0:gp, :], fold[0:gp, :], AF.Sqrt, bias=eps2[0:gp])
        nc.vector.reciprocal_approx_fast(r4[0:gp, :], r4[0:gp, :])
        nc.vector.tensor_copy(rinv_f4_h[0:gp, k, :], r4[0:gp, :])

        # nat-layout rinv for the A-side hinge scales, via PE transpose:
        # rows 128c+p live at rinv_f4_h[32(c//3), k, 128(c%3)+p]; transposing
        # block c' puts them on partition p at column 32(c//3).
        nblk = min(3, cA)
        trp = tr_psum.tile([P, 3 * P], FP16, tag="trp")
        for cp in range(nblk):
            nc.tensor.transpose(trp[:, cp * P:(cp + 1) * P],
                                rinv_f4_h[:, k, cp * P:(cp + 1) * P],
                                ident[:])
        j0 = k * CNAT
        rf = aux[:, 0, :]
        trv = trp[:].rearrange("p (b q) -> p b q", q=P)
        nc.vector.tensor_copy(rf[:, j0:j0 + nblk], trv[:, 0:nblk, 0])
        if cA > 3:
            nc.vector.tensor_copy(rf[:, j0 + 3:j0 + cA],
                                  trv[:, 0:cA - 3, 32])
        tf = aux[:, 1, j0:j0 + CNAT]
        nc.vector.reciprocal_approx_fast(tf, aux[:, 0, j0:j0 + CNAT])
        nc.vector.tensor_scalar_mul(tf, tf, THRESH)          # t * r

    def phase_b(h):
        k0 = h * 4
        # assemble the B-range of the rinv rows on partition 0 (strip 0 is
        # always the A side: la >= 512 > WG)
        for g in range(1, 4):
            nc.scalar.dma_start(
                row_all[0:1, k0:k0 + 4, g * WG:(g + 1) * WG],
                rinv_f4_h[32 * g:32 * g + 1, k0:k0 + 4, :])

    ets_tiles = [None] * BPC

    def phase_bc(k):
        s = slots[k]
        cA, nB = s["cA"], s["nB"]
        la = cA * P
        rinv_bc = bpool.tile([P, 6 * P], FP16, tag="rbc")
        nc.gpsimd.partition_broadcast(
            rinv_bc[:, 0:nB], row_all[0:1, k, la:la + nB], channels=P)
        ets = bpool.tile([P, 6 * P], BF16, tag="ets")
        ets_tiles[k] = ets
        nc.vector.tensor_tensor(ets[:, 0:nB], ett_tiles[k][:, la:la + nB],
                                rinv_bc[:, 0:nB], ALU.mult)

    def phase_c(k, cidx, hidx):
        s = slots[k]
        cA, nB = s["cA"], s["nB"]
        ets = ets_tiles[k]
        ett = ett_tiles[k]
        for c in range(cA):
            sim = sim_psum.tile([P, 1024], F32, tag="sim")
            for j0 in range(0, nB, 512):
                jw = min(512, nB - j0)
                nc.tensor.matmul(sim[:, j0:j0 + jw],
                                 lhsT=ett[:, c * P:(c + 1) * P],
                                 rhs=ets[:, j0:j0 + jw],
                                 start=True, stop=True)
            j = k * CNAT + c
            if hidx % DVE_EVERY == DVE_EVERY - 1:
                nc.vector.tensor_scalar(sim[:, 0:nB], sim[:, 0:nB],
                                        aux[:, 1, j:j + 1], None,
                                        ALU.max, ALU.add,
                                        accum_out=slot_all[:, cidx:cidx + 1])
            else:
                nc.scalar.activation(sim[:, 0:nB], sim[:, 0:nB], AF.Relu,
                                     bias=negt[:],
                                     scale=aux[:, 0, j:j + 1],
                                     accum_out=slot_all[:, cidx:cidx + 1])
            cidx += 1
            hidx += 1
        return cidx, hidx

    for k in range(BPC):
        load_slot(k)
    sq_fold_slot(0)
    sq_fold_slot(1)
    for k in range(4):
        if k + 2 < BPC:
            sq_fold_slot(k + 2)
        phase_a(k)
    phase_b(0)
    for k in range(4):
        phase_bc(k)
    for k in range(4, 8):
        if k + 2 < BPC:
            sq_fold_slot(k + 2)
        phase_a(k)
    phase_b(1)
    for k in range(4, 8):
        phase_bc(k)
    cidx = hidx = 0
    for k in range(BPC):
        cidx, hidx = phase_c(k, cidx, hidx)

    nc.sync.dma_start(out_ap[:, :], slot_all[:])
    nc.sync.dma_start(rout_ap[:, :], aux[:].rearrange("p a j -> p (a j)"))


_NC_CACHE = {}


def _build(meta_key, meta):
    if meta_key in _NC_CACHE:
        return _NC_CACHE[meta_key]
    nc = bacc.Bacc("TRN2", target_bir_lowering=False, debug=False,
                   num_devices=NCORES)
    ett = nc.dram_tensor("ett", [P, meta["lt"]], BF16, kind="ExternalInput")
    out = nc.dram_tensor("out", [P, meta["c_total"]], F32,
                         kind="ExternalOutput")
    rout = nc.dram_tensor("rout", [P, 2 * BPC * CNAT], F32,
                          kind="ExternalOutput")
    with tile.TileContext(nc) as tc:
        with ExitStack() as ctx:
            _kernel_body(ctx, tc, ett.ap(), out.ap(), rout.ap(), meta)
    nc.compile()
    _NC_CACHE[meta_key] = nc
    return nc


def _plan(labels):
    """Slot-sort samples and compute per-slot static shapes + assignments."""
    npos = (labels == 1).sum(axis=1)
    order = np.argsort(-npos, kind="stable")  # ranks by npos desc
    slots = []
    assign = np.zeros((NCORES, BPC), dtype=np.int64)  # -> original sample idx
    off = 0
    for k in range(BPC):
        members = order[k * NCORES:(k + 1) * NCORES]
        for c in range(NCORES):
            assign[c, k] = members[c]
        npk = max(int(npos[members].max()), 1)
        nnk = max(int((N - npos[members]).max()), 1)
        # orientation: stationary A side = whichever gives fewer chunk-ops
        cost_pos = -(-npk // P) * nnk
        cost_neg = -(-nnk // P) * npk
        a_is_pos = cost_pos <= cost_neg
        na, nb = (npk, nnk) if a_is_pos else (nnk, npk)
        ca = -(-na // P)
        lpad = -(-(ca * P + nb) // WG) * WG
        slots.append({"cA": ca, "nB": nb, "off": off, "lpad": lpad,
                      "a_is_pos": a_is_pos})
        off += lpad
    c_total = sum(s["cA"] for s in slots)
    lmax = max(s["lpad"] for s in slots)
    meta = {"slots": slots, "lt": off, "c_total": c_total, "lmax": lmax}
    return meta, assign


def kernel(embeddings: np.ndarray, labels: np.ndarray,
           _want_results=False, _trace=False) -> np.ndarray:
    emb = np.ascontiguousarray(embeddings, dtype=np.float32)
    lab = np.asarray(labels)
    assert emb.shape == (B, N, D) and lab.shape == (B, N)

    meta, assign = _plan(lab)
    slots = meta["slots"]

    # pack: per (core, slot) transposed bf16 [128, L] = [A-cols | B-cols]
    packed = np.zeros((NCORES, P, meta["lt"]), dtype=ml_dtypes.bfloat16)
    for c in range(NCORES):
        for k, s in enumerate(slots):
            b = assign[c, k]
            pos_idx = np.nonzero(lab[b] == 1)[0]
            neg_idx = np.nonzero(lab[b] == 0)[0]
            if not s["a_is_pos"]:
                pos_idx, neg_idx = neg_idx, pos_idx
            et = emb[b].T.astype(ml_dtypes.bfloat16)
            off, la = s["off"], s["cA"] * P
            packed[c, :, off:off + len(pos_idx)] = et[:, pos_idx]
            packed[c, :, off + la:off + la + len(neg_idx)] = et[:, neg_idx]

    key = (meta["lt"], tuple((s["cA"], s["nB"], s["a_is_pos"]) for s in slots))
    nc = _build(key, meta)
    in_maps = [{"ett": packed[c]} for c in range(NCORES)]
    res = bass_utils.run_bass_kernel_spmd(nc, in_maps,
                                          core_ids=list(range(NCORES)),
                                          trace=_trace)

    # host combine: exact unscale + corrections + final division
    loss_sum = 0.0
    count = 0.0
    npos_all = (lab == 1).sum(axis=1)
    nneg_all = (lab == 0).sum(axis=1)
    for c in range(NCORES):
        slot_out = np.asarray(res.results[c]["out"], dtype=np.float64)
        aux_out = np.asarray(res.results[c]["rout"],
                             dtype=np.float64).reshape(P, 2, BPC, CNAT)
        rinv_out = aux_out[:, 0]
        thr_out = aux_out[:, 1]
        cidx = 0
        hidx = 0
        for k, s in enumerate(slots):
            b = assign[c, k]
            np_, nn_ = int(npos_all[b]), int(nneg_all[b])
            sb = 0.0
            for ch in range(s["cA"]):
                col = slot_out[:, cidx]
                if hidx % DVE_EVERY == DVE_EVERY - 1:
                    sb += float(((col - s["nB"] * thr_out[:, k, ch])
                                 * rinv_out[:, k, ch]).sum())
                else:
                    sb += float(col.sum())
                cidx += 1
                hidx += 1
            if np_ > 0 and nn_ > 0:
                loss_sum += sb / max(nn_, 1)
                count += np_
    ans = np.float32(np.float32(loss_sum) / np.float32(max(count, 1.0)))
    if _want_results:
        return ans, res
    return ans
